# revision 30
# baseline (speedup 1.0000x reference)
import sys
sys.path.insert(0, "/opt/trn_rl_repo")
import numpy as np
import ml_dtypes

from concourse import bass, bacc, mybir, tile
from concourse.bass_utils import run_bass_kernel_spmd

F32 = mybir.dt.float32
BF16 = mybir.dt.bfloat16
I16 = mybir.dt.int16

N, E_TOT, DIN, HID, HEADS = 50000, 800000, 128, 64, 4
NC = 8
NPC = N // NC          # 6250 nodes per core
BS = 125               # dst nodes per block
NB = NPC // BS         # 50 blocks per core
SPLIT = 32768          # int16 index split for source tables
R1 = 384               # T1 row: 256 bf16 h1 + f32 ar(4)/g(1) at f32 idx 128..133 (768B)
R2 = 128               # T2 row: 64 bf16 h2 + f32 ar2/g2 at f32 idx 32..34 (256B)
NEG = 0.01
FGW = 0.1
PADOFF = 999.0
SCRATCH = 16384        # SWDGE ring: 1024 descs (HW ucode limit: >1024 idxs per call crashes)
CH = SCRATCH // 16 // 128
SBV = 10               # blocks per idx-table streaming superblock (even)


def _wrap_idx(arr):
    # dma_gather wrapped layout: idx i -> [i%16, i//16], replicated on 8 groups
    L = len(arr)
    w = arr.reshape(L // 16, 16).T
    return np.tile(w, (8, 1)).astype(np.int16)


def _prep_edges(src, dst):
    """Per-core edge layout. Returns per-core dicts + shared tile counts."""
    cores = []
    for c in range(NC):
        base = c * NPC
        m = (dst >= base) & (dst < base + NPC)
        s, d = src[m], dst[m] - base
        blk = d // BS
        doff = d % BS
        per_blk = []
        for b in range(NB):
            bm = blk == b
            bs, bdo = s[bm], doff[bm]
            a = bs < SPLIT
            per_blk.append(
                (bs[a], bdo[a], bs[~a] - SPLIT, bdo[~a])
            )
        cores.append(per_blk)
    TA = [max(int(np.ceil(len(cores[c][b][0]) / 128)) for c in range(NC)) for b in range(NB)]
    TB = [max(int(np.ceil(len(cores[c][b][2]) / 128)) for c in range(NC)) for b in range(NB)]
    TT = [TA[b] + TB[b] for b in range(NB)]
    Tm = sum(TT)
    outs = []
    for c in range(NC):
        idxm = np.zeros(Tm * 128, np.int16)
        idxal = np.zeros(Tm * 128, np.int16)
        doffm = np.full((Tm, 128), PADOFF, np.float32)
        o = 0
        for b in range(NB):
            sa, da, sb_, db = cores[c][b]
            for (ss, dd, tcount) in ((sa, da, TA[b]), (sb_, db, TB[b])):
                n = len(ss)
                idxm[o:o + n] = ss.astype(np.int16)
                idxal[o:o + n] = (dd + b * BS).astype(np.int16)
                fl_view = np.full(tcount * 128, PADOFF, np.float32)
                fl_view[:n] = dd.astype(np.float32)
                doffm[o // 128:o // 128 + tcount] = fl_view.reshape(tcount, 128)
                o += tcount * 128
        outs.append({
            "idxm": _wrap_idx(idxm),
            "idxal": _wrap_idx(idxal),
            "doffm": doffm.T.astype(ml_dtypes.bfloat16).copy(),  # [128, Tm]
        })
    return outs, TA, TB, TT, Tm


def _build_program(TA, TB, TT, Tm, upto=6, nb_limit=None, ep_flags=("g1", "g2", "gal", "mm", "out"), siminit=False, no_collective=False, simlrelu=True):
    # upto: 1=A, 3=+AG1+L1 edges, 5=+D+AG2, 6=+L2 (full)
    # siminit: fully init tile tails so CoreSim's uninit-read check passes
    nb_lim = NB if nb_limit is None else nb_limit
    ep_flags = frozenset(ep_flags)
    nc = bacc.Bacc(dynamic_dma_scratch_size=SCRATCH)
    xTl = nc.declare_dram_parameter("xTl", [DIN, NPC], F32, isOutput=False)
    W1e = nc.declare_dram_parameter("W1e", [DIN, 265], F32, isOutput=False)
    W2e = nc.declare_dram_parameter("W2e", [256, 67], BF16, isOutput=False)
    W2al = nc.declare_dram_parameter("W2al", [256, 1], BF16, isOutput=False)
    fcw = nc.declare_dram_parameter("fcw", [128, HID], F32, isOutput=False)
    fcb = nc.declare_dram_parameter("fcb", [128, 1], F32, isOutput=False)
    iota = nc.declare_dram_parameter("iota", [128, 128], BF16, isOutput=False)
    gl = nc.declare_dram_parameter("gl", [128, (NPC + 127) // 128], F32, isOutput=False)
    idxm = nc.declare_dram_parameter("idxm", [128, Tm * 8], I16, isOutput=False)
    idxal = nc.declare_dram_parameter("idxal", [128, Tm * 8], I16, isOutput=False)
    doffm = nc.declare_dram_parameter("doffm", [128, Tm], BF16, isOutput=False)
    yout = nc.declare_dram_parameter("yout", [NPC, 1], F32, isOutput=True)

    T1own = nc.dram_tensor("T1own", [NPC, R1], BF16)
    T2own = nc.dram_tensor("T2own", [NPC, R2], BF16)
    al1 = nc.dram_tensor("al1", [NPC, 64], F32)
    al2 = nc.dram_tensor("al2", [NPC, 64], F32)
    if no_collective:
        T1all = nc.dram_tensor("T1all", [N, R1], BF16)
        T2all = nc.dram_tensor("T2all", [N, R2], BF16)
    else:
        T1all = nc.dram_tensor("T1all", [N, R1], BF16, addr_space="Shared")
        T2all = nc.dram_tensor("T2all", [N, R2], BF16, addr_space="Shared")

    with tile.TileContext(nc) as tc:
        # ---------- resident constants ----------
        with tc.tile_pool(name="const", bufs=1) as cp:
            W1e_sb = cp.tile([DIN, 265], F32, tag="w1e")
            nc.sync.dma_start(out=W1e_sb[:], in_=W1e[:, :])
            W2e_sb = []
            for k in range(2):
                t_ = cp.tile([128, 67], BF16, tag=f"w2e{k}")
                nc.sync.dma_start(out=t_[:], in_=W2e[k * 128:(k + 1) * 128, :])
                W2e_sb.append(t_)
            W2al_sb = []
            for k in range(2):
                t_ = cp.tile([128, 1], BF16, tag=f"w2al{k}")
                nc.sync.dma_start(out=t_[:], in_=W2al[k * 128:(k + 1) * 128, :])
                W2al_sb.append(t_)
            fcw_sb = cp.tile([128, HID], F32, tag="fcw")
            nc.sync.dma_start(out=fcw_sb[:], in_=fcw[:, :])
            fcb_sb = cp.tile([128, 1], F32, tag="fcb")
            nc.sync.dma_start(out=fcb_sb[:], in_=fcb[:, :])
            iota_sb = cp.tile([128, 128], BF16, tag="iota")
            nc.sync.dma_start(out=iota_sb[:], in_=iota[:, :])
            doffm_sb = cp.tile([128, Tm], BF16, tag="doffm")
            nc.sync.dma_start(out=doffm_sb[:], in_=doffm[:, :])
            from concourse.masks import make_identity
            ident_sb = cp.tile([128, 128], BF16, tag="ident")
            make_identity(nc, ident_sb[:])
            eps_sb = cp.tile([128, 1], F32, tag="epsc")
            nc.vector.memset(eps_sb[:], 1e-6)
            gl_sb = cp.tile([128, (NPC + 127) // 128], F32, tag="gl")
            nc.sync.dma_start(out=gl_sb[:], in_=gl[:, :])

            # ---------- phase A: build own-shard T1 + al1, then AllGather ----
            FA = 4
            ntileA = (NPC + 127) // 128
            with tc.tile_pool(name="bA", bufs=3) as bp, \
                 tc.tile_pool(name="bAp", bufs=4, space="PSUM") as bpp:
                for i0 in range(0, ntileA if upto >= 1 else 0, FA):
                    nf = min(FA, ntileA - i0)
                    s0 = i0 * 128
                    mg = min(nf * 128, NPC - s0)
                    xs = bp.tile([DIN, FA * 128], F32, tag="xs")
                    nc.sync.dma_start(out=xs[:, :mg], in_=xTl[:, s0:s0 + mg])
                    row = bp.tile([128, FA * R1], BF16, tag="rowA")
                    alt = bp.tile([128, FA * 64], F32, tag="altA")
                    if siminit:
                        nc.vector.memset(row[:], 0.0)
                        nc.vector.memset(alt[:], 0.0)
                    rowf = row[:].bitcast(F32)
                    for j in range(nf):
                        s = s0 + j * 128
                        m = min(128, NPC - s)
                        ps = bpp.tile([128, 265], F32, space="PSUM", tag="psA")
                        nc.tensor.matmul(out=ps[:m, :],
                                         lhsT=xs[:, j * 128:j * 128 + m],
                                         rhs=W1e_sb[:], start=True, stop=True)
                        co = j * R1
                        cof = j * (R1 // 2)
                        nc.vector.tensor_copy(out=row[:m, co:co + 256],
                                              in_=ps[:m, 0:256])
                        nc.vector.tensor_copy(out=rowf[:m, cof + 128:cof + 132],
                                              in_=ps[:m, 256:260])
                        nc.vector.tensor_copy(out=alt[:m, j * 64:j * 64 + 4],
                                              in_=ps[:m, 260:264])
                        nc.vector.tensor_copy(out=rowf[:m, cof + 132:cof + 133],
                                              in_=gl_sb[:m, i0 + j:i0 + j + 1])
                    tfull = mg // 128
                    if tfull:
                        nc.sync.dma_start(
                            out=T1own[s0:s0 + tfull * 128, :].rearrange(
                                "(t p) e -> p t e", p=128),
                            in_=row[:, :tfull * R1].rearrange("p (t e) -> p t e", e=R1))
                        nc.sync.dma_start(
                            out=al1[s0:s0 + tfull * 128, 0:8].rearrange(
                                "(t p) e -> p t e", p=128),
                            in_=alt[:, :tfull * 64].rearrange(
                                "p (t e) -> p t e", e=64)[:, :, 0:8])
                    rem = mg - tfull * 128
                    if rem:
                        nc.sync.dma_start(
                            out=T1own[s0 + tfull * 128:s0 + mg, :],
                            in_=row[:rem, tfull * R1:(tfull + 1) * R1])
                        nc.sync.dma_start(
                            out=al1[s0 + tfull * 128:s0 + mg, 0:8],
                            in_=alt[:rem, tfull * 64:tfull * 64 + 8])

            if upto >= 3 and not no_collective:
                nc.gpsimd.collective_compute(
                    "AllGather", mybir.AluOpType.bypass,
                    replica_groups=[list(range(NC))],
                    ins=[T1own[:, :].opt()], outs=[T1all[:, :].opt()])

            # ---------- shared edge pass ----------
            def edge_pass(tab, altab, rowlen, mw, aroff, out_cb, name,
                          post_block=None, al_pe=None, depth=None,
                          pmx_bufs=None, alE_bufs=1):
                # rowlen bf16 elems; mw = msg width (256 or 64); heads = mw//64
                # al_pe: [128, NB] bf16 per-block dst-al table. When set, the
                # per-edge al values are built ON-CHIP (PE transpose of sel +
                # a 1-col matmul per tile) instead of SWDGE-gathered — saves
                # ~155us of DMA and ~125 Pool calls per layer. The scatter is
                # then deferred one block so the PE queue never stalls on the
                # current block's message chain.
                H = mw // 64
                rf = rowlen // 2
                offs = [0]
                for b in range(nb_lim):
                    offs.append(offs[-1] + TT[b] * 128)
                sbcols = max(
                    (offs[min(b0 + SBV, nb_lim)] - offs[b0]) // 16
                    for b0 in range(0, nb_lim, SBV)) if nb_lim else 8
                from contextlib import ExitStack
                with ExitStack() as stk:
                    epg = stk.enter_context(tc.tile_pool(name=f"e{name}g", bufs=3))
                    ep = stk.enter_context(tc.tile_pool(
                        name=f"e{name}",
                        bufs=(depth or (3 if al_pe is not None else 2))))
                    epi = stk.enter_context(tc.tile_pool(name=f"e{name}i", bufs=2))
                    eps = stk.enter_context(tc.tile_pool(name=f"e{name}s", bufs=4))
                    epp = stk.enter_context(tc.tile_pool(
                        name=f"e{name}p",
                        bufs=(pmx_bufs or (2 if al_pe is not None else 3)),
                        space="PSUM"))
                    if al_pe is not None:
                        eppT = stk.enter_context(
                            tc.tile_pool(name=f"e{name}pT", bufs=1, space="PSUM"))
                        eppA = stk.enter_context(
                            tc.tile_pool(name=f"e{name}pA", bufs=alE_bufs,
                                         space="PSUM"))

                    def chunked_gather(out_full, src_ap, idx_sb, o_base, ntiles, elem):
                        done = 0
                        while done < ntiles:
                            ch = min(CH, ntiles - done)
                            i0 = o_base + done * 128
                            nc.gpsimd.dma_gather(
                                out_ap=out_full[:, done * elem:(done + ch) * elem
                                                ].rearrange("p (t e) -> p t e", e=elem),
                                in_ap=src_ap,
                                idxs_ap=idx_sb[:, i0 // 16:(i0 + ch * 128) // 16],
                                num_idxs=ch * 128, num_idxs_reg=ch * 128,
                                elem_size=elem)
                            done += ch

                    def build_sel(o, t_all):
                        sel_all = ep.tile([128, t_all * 128], BF16, tag="sel")
                        nc.vector.tensor_tensor(
                            out=sel_all[:].rearrange("p (t j) -> p t j", j=128),
                            in0=doffm_sb[:, o // 128:o // 128 + t_all
                                         ][:, :, None].to_broadcast(
                                             [128, t_all, 128]),
                            in1=iota_sb[:][:, None, :].to_broadcast(
                                [128, t_all, 128]),
                            op=mybir.AluOpType.is_equal)
                        return sel_all

                    def scatter(sel_all, msg, t_all, mwx):
                        pmx = epp.tile([128, mwx], F32, space="PSUM", tag="pm")
                        for t in range(t_all):
                            nc.tensor.matmul(out=pmx[:BS, :],
                                             lhsT=sel_all[:, t * 128:t * 128 + BS],
                                             rhs=msg[:, t * mwx:(t + 1) * mwx],
                                             start=(t == 0), stop=(t == t_all - 1))
                        return pmx

                    o = 0
                    Galp = None
                    gal_off = 0
                    idxm_t = idxal_t = None
                    sb_base = 0
                    pend = None
                    for b in range(nb_lim):
                        if b % SBV == 0:
                            # stream the idx tables per superblock: resident
                            # copies of the full tables would cost 27KB of SBUF
                            sb_end = min(b + SBV, nb_lim)
                            c0, c1 = offs[b] // 16, offs[sb_end] // 16
                            idxm_t = epi.tile([128, sbcols], I16, tag="idxm")
                            nc.sync.dma_start(out=idxm_t[:, :c1 - c0],
                                              in_=idxm[:, c0:c1])
                            if al_pe is None:
                                idxal_t = epi.tile([128, sbcols], I16, tag="idxal")
                                nc.sync.dma_start(out=idxal_t[:, :c1 - c0],
                                                  in_=idxal[:, c0:c1])
                            sb_base = offs[b]
                        ta, tb_, t_all = TA[b], TB[b], TT[b]
                        G = epg.tile([128, t_all * rowlen], BF16, tag="G")
                        if ta and "g1" in ep_flags:
                            chunked_gather(G[:, :ta * rowlen], tab[:, :], idxm_t,
                                           o - sb_base, ta, rowlen)
                        elif ta:
                            nc.vector.memset(G[:, :ta * rowlen], 0.25)
                        if tb_ and "g2" in ep_flags:
                            chunked_gather(G[:, ta * rowlen:t_all * rowlen],
                                           tab[SPLIT:, :], idxm_t,
                                           o - sb_base + ta * 128, tb_, rowlen)
                        elif tb_:
                            nc.vector.memset(G[:, ta * rowlen:t_all * rowlen], 0.25)
                        sel_all = None
                        if al_pe is None:
                            # al gather paired across 2 blocks: the idxal stream
                            # is contiguous, so one gather covers both (fewer
                            # ~1us fixed-cost SWDGE calls)
                            if b % 2 == 0:
                                tpair = t_all + (TT[b + 1] if b + 1 < nb_lim else 0)
                                Galp = ep.tile([128, tpair * 64], F32, tag="Gal")
                                if "gal" in ep_flags:
                                    chunked_gather(Galp[:], altab[:, :], idxal_t,
                                                   o - sb_base, tpair, 64)
                                else:
                                    nc.vector.memset(Galp[:], 0.25)
                                gal_off = 0
                            Gal = Galp[:, gal_off * 64:(gal_off + t_all) * 64]
                            gal_off += t_all
                            alE_in = Gal.rearrange("p (t q) -> p t q", q=64)[:, :, 0:H]
                        else:
                            sel_all = build_sel(o, t_all)
                            selTp = eppT.tile([128, t_all * 128], BF16,
                                              space="PSUM", tag="selT")
                            for t in range(t_all):
                                nc.tensor.transpose(
                                    out=selTp[:, t * 128:(t + 1) * 128],
                                    in_=sel_all[:, t * 128:(t + 1) * 128],
                                    identity=ident_sb[:, :])
                            selT_sb = ep.tile([128, t_all * 128], BF16, tag="selTs")
                            nc.scalar.activation(
                                out=selT_sb[:], in_=selTp[:],
                                func=mybir.ActivationFunctionType.Copy)
                            alEp = eppA.tile([128, t_all * H], F32,
                                             space="PSUM", tag="alE")
                            for t in range(t_all):
                                nc.tensor.matmul(
                                    out=alEp[:, t * H:(t + 1) * H],
                                    lhsT=selT_sb[:, t * 128:(t + 1) * 128],
                                    rhs=al_pe[:, b * H:(b + 1) * H],
                                    start=True, stop=True)
                            alE_in = alEp[:].rearrange("p (t h) -> p t h", h=H)
                        Gf = G[:].bitcast(F32).rearrange("p (t q) -> p t q", q=rf)
                        # alpha = lrelu(al+ar); e = exp(alpha + g)
                        a4 = eps.tile([128, t_all * H], F32, tag="a4")
                        a4v = a4[:].rearrange("p (t h) -> p t h", h=H)
                        nc.vector.tensor_tensor(
                            out=a4v,
                            in0=alE_in,
                            in1=Gf[:, :, aroff:aroff + H],
                            op=mybir.AluOpType.add)
                        if simlrelu:
                            a4s = eps.tile([128, t_all * H], F32, tag="a4s")
                            nc.vector.tensor_scalar_mul(out=a4s[:], in0=a4[:],
                                                        scalar1=NEG)
                            nc.vector.tensor_tensor(out=a4[:], in0=a4[:],
                                                    in1=a4s[:],
                                                    op=mybir.AluOpType.max)
                        else:
                            nc.scalar.activation(out=a4[:], in_=a4[:],
                                                 func=mybir.ActivationFunctionType.Lrelu,
                                                 alpha=NEG)
                        nc.vector.tensor_tensor(
                            out=a4v, in0=a4v,
                            in1=Gf[:, :, aroff + H:aroff + H + 1].to_broadcast(
                                [128, t_all, H]),
                            op=mybir.AluOpType.add)
                        # exp on Activation, straight to bf16
                        eb = eps.tile([128, t_all * H], BF16, tag="eb")
                        nc.scalar.activation(out=eb[:], in_=a4[:],
                                             func=mybir.ActivationFunctionType.Exp)
                        # expand eb across channels on Activation (idle engine);
                        # the big multiply then runs packed on DVE (2x mode)
                        ebx = ep.tile([128, t_all * mw], BF16, tag="ebx")
                        nc.scalar.activation(
                            out=ebx[:].rearrange("p (t h d) -> p t h d", h=H, d=64),
                            in_=eb[:].rearrange("p (t h) -> p t h", h=H)[
                                :, :, :, None].to_broadcast([128, t_all, H, 64]),
                            func=mybir.ActivationFunctionType.Copy)
                        # msg = [h * e | e]: eb rides as H extra columns so ONE
                        # matmul per tile scatters numerator + denominator
                        mwx = mw + H
                        msg = ep.tile([128, t_all * mwx], BF16, tag="msg")
                        msgv = msg[:].rearrange("p (t c) -> p t c", c=mwx)
                        nc.vector.tensor_tensor(
                            out=msgv[:, :, 0:mw],
                            in0=G[:].rearrange("p (t q) -> p t q", q=rowlen)[
                                :, :, 0:mw],
                            in1=ebx[:].rearrange("p (t c) -> p t c", c=mw),
                            op=mybir.AluOpType.mult)
                        nc.scalar.activation(
                            out=msgv[:, :, mw:mwx],
                            in_=eb[:].rearrange("p (t h) -> p t h", h=H),
                            func=mybir.ActivationFunctionType.Copy)
                        if "mm" in ep_flags:
                            if al_pe is None:
                                sel_all = build_sel(o, t_all)
                                pmx = scatter(sel_all, msg, t_all, mwx)
                                if "out" in ep_flags:
                                    out_cb(b, pmx[:, 0:mw], pmx[:, mw:mwx], eps)
                            else:
                                # deferred scatter: emit block b-1's scatter
                                # now so the PE queue's next entries (block
                                # b+1 transposes) never wait on block b's
                                # message chain
                                if pend is not None:
                                    pb_, psel, pmsg, pt_all = pend
                                    pmx = scatter(psel, pmsg, pt_all, mwx)
                                    if "out" in ep_flags:
                                        out_cb(pb_, pmx[:, 0:mw],
                                               pmx[:, mw:mwx], eps)
                                pend = (b, sel_all, msg, t_all)
                        else:
                            pmx = epp.tile([128, mwx], F32, space="PSUM", tag="pm")
                            nc.vector.memset(pmx[:BS, :], 0.5)
                            if "out" in ep_flags:
                                out_cb(b, pmx[:, 0:mw], pmx[:, mw:mwx], eps)
                        if post_block is not None:
                            post_block(b)
                        o += t_all * 128
                    if pend is not None:
                        pb_, psel, pmsg, pt_all = pend
                        mwx = mw + H
                        pmx = scatter(psel, pmsg, pt_all, mwx)
                        if "out" in ep_flags:
                            out_cb(pb_, pmx[:, 0:mw], pmx[:, mw:mwx], eps)

            # ---------- phase B: layer-1 edges ----------
            # z1 accumulates in SBUF across all blocks; feeds phase D directly
            # (no DRAM roundtrip for own-shard z).
            zT_sb = cp.tile([128, 2 * NPC], BF16, tag="zTsb")
            ytile = cp.tile([128, NB], F32, tag="ytile")

            def l1_out(b, pm, psv, eps):
                r4 = eps.tile([128, HEADS], F32, tag="r4")
                nc.vector.tensor_scalar_add(out=r4[:BS], in0=psv[:BS, :], scalar1=1e-16)
                nc.vector.reciprocal(out=r4[:BS], in_=r4[:BS])
                z = eps.tile([128, 256], F32, tag="z")
                nc.vector.tensor_tensor(
                    out=z[:BS].rearrange("p (h d) -> p h d", d=64),
                    in0=pm[:BS, :].rearrange("p (h d) -> p h d", d=64),
                    in1=r4[:BS, :, None].to_broadcast([BS, HEADS, 64]),
                    op=mybir.AluOpType.mult)
                # ELU
                m1 = eps.tile([128, 256], F32, tag="m1")
                nc.vector.tensor_scalar_min(out=m1[:BS], in0=z[:BS], scalar1=0.0)
                nc.scalar.activation(out=m1[:BS], in_=m1[:BS],
                                     func=mybir.ActivationFunctionType.Exp)
                p1 = eps.tile([128, 256], F32, tag="p1")
                nc.vector.tensor_scalar(out=p1[:BS], in0=z[:BS], scalar1=0.0,
                                        scalar2=1.0, op0=mybir.AluOpType.max,
                                        op1=mybir.AluOpType.subtract)
                zb = eps.tile([128, 256], BF16, tag="zb")
                nc.vector.tensor_tensor(out=zb[:BS], in0=m1[:BS], in1=p1[:BS],
                                        op=mybir.AluOpType.add)
                for k in range(2):
                    pt = epp_t.tile([128, BS], BF16, space="PSUM", tag="pt")
                    nc.tensor.transpose(out=pt[:, :BS], in_=zb[:BS, k * 128:(k + 1) * 128],
                                        identity=ident_sb[:BS, :BS])
                    nc.vector.tensor_copy(
                        out=zT_sb[:, k * NPC + b * BS:k * NPC + (b + 1) * BS],
                        in_=pt[:, :BS])

            # ---------- phase D: own-shard T2 + al2 from SBUF-resident z ----
            # Issued INSIDE the L1 edge pass: D-group g only needs the zT_sb
            # columns written by the first ~(g*FD*128/BS) L1 blocks, so the
            # build overlaps L1's pipeline instead of serializing after it.
            FD = 8
            ntileD = (NPC + 127) // 128

            if upto >= 3:
                al1blk = cp.tile([128, NB * HEADS], BF16, tag="al1blk")
                nc.vector.memset(al1blk[:], 0.0)
                al1st = cp.tile([128, NB * HEADS], F32, tag="al1st")
                nc.sync.dma_start(
                    out=al1st[:BS, :].rearrange("p (b e) -> p b e", e=HEADS),
                    in_=al1[:, 0:HEADS].rearrange("(b p) e -> p b e", p=BS))
                nc.vector.tensor_copy(out=al1blk[:BS, :], in_=al1st[:BS, :])
            with tc.tile_pool(name="tp", bufs=1, space="PSUM") as epp_t:
                if upto >= 3:
                    edge_pass(T1all, al1, R1, 256, 128, l1_out, "L1",
                              al_pe=al1blk, alE_bufs=2)
            with tc.tile_pool(name="bD", bufs=3) as bpD, \
                 tc.tile_pool(name="bDp", bufs=2, space="PSUM") as bppD:

                def issue_d_group(g):
                    i0 = g * FD
                    nf = min(FD, ntileD - i0)
                    s0 = i0 * 128
                    mg = min(nf * 128, NPC - s0)
                    row = bpD.tile([128, FD * R2], BF16, tag="rowD")
                    if siminit:
                        nc.vector.memset(row[:], 0.0)
                    rowf = row[:].bitcast(F32)
                    psg = bppD.tile([128, 296], F32, space="PSUM", tag="psD")
                    for j in range(nf):
                        s = s0 + j * 128
                        m = min(128, NPC - s)
                        ps = psg[:, (j % 4) * 72:(j % 4) * 72 + 67]
                        for k in range(2):
                            nc.tensor.matmul(
                                out=ps[:m, :],
                                lhsT=zT_sb[:, k * NPC + s:k * NPC + s + m],
                                rhs=W2e_sb[k][:],
                                start=(k == 0), stop=(k == 1))
                        co = j * R2
                        cof = j * (R2 // 2)
                        nc.vector.tensor_copy(out=row[:m, co:co + 64],
                                              in_=ps[:m, 0:64])
                        nc.vector.tensor_copy(out=rowf[:m, cof + 32:cof + 33],
                                              in_=ps[:m, 64:65])
                        ab = bpD.tile([128, 1], F32, tag="absD")
                        nc.vector.tensor_scalar_mul(out=ab[:m], in0=ps[:m, 66:67],
                                                    scalar1=-1.0)
                        nc.vector.tensor_tensor(out=ab[:m], in0=ab[:m],
                                                in1=ps[:m, 66:67],
                                                op=mybir.AluOpType.max)
                        ln = bpD.tile([128, 1], F32, tag="lnD")
                        nc.scalar.activation(out=ln[:m], in_=ab[:m],
                                             func=mybir.ActivationFunctionType.Ln,
                                             bias=eps_sb[:m, 0:1])
                        nc.vector.tensor_scalar_mul(
                            out=rowf[:m, cof + 33:cof + 34],
                            in0=ln[:m], scalar1=-FGW)
                    tfull = mg // 128
                    if tfull:
                        nc.sync.dma_start(
                            out=T2own[s0:s0 + tfull * 128, :].rearrange(
                                "(t p) e -> p t e", p=128),
                            in_=row[:, :tfull * R2].rearrange(
                                "p (t e) -> p t e", e=R2))
                    rem = mg - tfull * 128
                    if rem:
                        nc.sync.dma_start(
                            out=T2own[s0 + tfull * 128:s0 + mg, :],
                            in_=row[:rem, tfull * R2:(tfull + 1) * R2])
                    # al2 for this group
                    ps2 = psg[:, 288:296]
                    for j in range(nf):
                        s = s0 + j * 128
                        m = min(128, NPC - s)
                        for k in range(2):
                            nc.tensor.matmul(out=ps2[:m, j:j + 1],
                                             lhsT=zT_sb[:, k * NPC + s:
                                                        k * NPC + s + m],
                                             rhs=W2al_sb[k][:],
                                             start=(k == 0), stop=(k == 1))
                    alt = bpD.tile([128, FD * 64], F32, tag="altD")
                    if siminit:
                        nc.vector.memset(alt[:], 0.0)
                    for j in range(nf):
                        m = min(128, NPC - s0 - j * 128)
                        nc.vector.tensor_copy(out=alt[:m, j * 64:j * 64 + 1],
                                              in_=ps2[:m, j:j + 1])
                    if tfull:
                        nc.sync.dma_start(
                            out=al2[s0:s0 + tfull * 128, 0:8].rearrange(
                                "(t p) e -> p t e", p=128),
                            in_=alt[:, :tfull * 64].rearrange(
                                "p (t e) -> p t e", e=64)[:, :, 0:8])
                    if rem:
                        nc.sync.dma_start(
                            out=al2[s0 + tfull * 128:s0 + mg, 0:8],
                            in_=alt[:rem, tfull * 64:tfull * 64 + 8])

                # serial D after L1: interleaving it into L1 loses ~45us to
                # in-order Act/SP queue head-of-line stalls on the D deps
                if upto >= 5:
                    for g in range((ntileD + FD - 1) // FD):
                        issue_d_group(g)

            if upto >= 5 and not no_collective:
                nc.gpsimd.collective_compute(
                    "AllGather", mybir.AluOpType.bypass,
                    replica_groups=[list(range(NC))],
                    ins=[T2own[:, :].opt()], outs=[T2all[:, :].opt()])

            # ---------- phase E: layer-2 edges + head ----------
            def l2_out(b, pm, psv, eps):
                r1 = eps.tile([128, 1], F32, tag="r1o")
                nc.vector.tensor_scalar_add(out=r1[:BS], in0=psv[:BS, 0:1], scalar1=1e-16)
                nc.vector.reciprocal(out=r1[:BS], in_=r1[:BS])
                z = eps.tile([128, 64], F32, tag="zo")
                nc.vector.tensor_tensor(out=z[:BS], in0=pm[:BS, :],
                                        in1=r1[:BS, 0:1].to_broadcast([BS, 64]),
                                        op=mybir.AluOpType.mult)
                m1 = eps.tile([128, 64], F32, tag="m1o")
                nc.vector.tensor_scalar_min(out=m1[:BS], in0=z[:BS], scalar1=0.0)
                nc.scalar.activation(out=m1[:BS], in_=m1[:BS],
                                     func=mybir.ActivationFunctionType.Exp)
                p1 = eps.tile([128, 64], F32, tag="p1o")
                nc.vector.tensor_scalar(out=p1[:BS], in0=z[:BS], scalar1=0.0,
                                        scalar2=1.0, op0=mybir.AluOpType.max,
                                        op1=mybir.AluOpType.subtract)
                nc.vector.tensor_tensor(out=z[:BS], in0=m1[:BS], in1=p1[:BS],
                                        op=mybir.AluOpType.add)
                nc.vector.tensor_tensor(out=z[:BS], in0=z[:BS], in1=fcw_sb[:BS, :],
                                        op=mybir.AluOpType.mult)
                y = eps.tile([128, 1], F32, tag="yo")
                nc.vector.tensor_reduce(out=y[:BS], in_=z[:BS],
                                        op=mybir.AluOpType.add,
                                        axis=mybir.AxisListType.X)
                nc.vector.tensor_scalar_add(out=ytile[:BS, b:b + 1], in0=y[:BS],
                                            scalar1=fcb_sb[:BS, 0:1])

            if upto >= 6:
                # per-block dst-al table for the PE-side al path: al2[d] for
                # block b sits at column b, partition d (rows 125..127 zero)
                al2blk = cp.tile([128, NB], BF16, tag="al2blk")
                nc.vector.memset(al2blk[:], 0.0)
                al2st = cp.tile([128, NB], F32, tag="al2st")
                nc.sync.dma_start(
                    out=al2st[:BS, :, None],
                    in_=al2[:, 0:1].rearrange("(b p) e -> p b e", p=BS))
                nc.vector.tensor_copy(out=al2blk[:BS, :], in_=al2st[:BS, :])
                edge_pass(T2all, al2, R2, 64, 32, l2_out, "L2", al_pe=al2blk,
                          pmx_bufs=3, alE_bufs=2)
                nc.sync.dma_start(
                    out=yout[:, :].rearrange("(b p) e -> p b e", p=BS),
                    in_=ytile[:BS, :][:, :, None])

    nc.compile()
    return nc


_CACHE = {}


def prep_all(x, src, dst, W1, al1_, ar1_, W2, al2_, ar2_, fc_w, fc_b):
    """Host-side prep: returns (in_maps, nc). Raises on compile failure."""
    per_core, TA, TB, TT, Tm = _prep_edges(src, dst)

    # fused weights
    W1e = np.zeros((DIN, 265), np.float32)
    W1e[:, 0:256] = W1
    for h in range(HEADS):
        W1e[:, 256 + h] = W1[:, h * HID:(h + 1) * HID] @ ar1_[h]
        W1e[:, 260 + h] = W1[:, h * HID:(h + 1) * HID] @ al1_[h]
    W1e[127, 264] = 1.0
    W2e = np.zeros((256, 67), np.float32)
    W2e[:, 0:64] = W2
    W2e[:, 64] = W2 @ ar2_[0]
    W2e[:, 65] = W2 @ al2_[0]
    W2e[255, 66] = 1.0
    W2al = (W2 @ al2_[0]).reshape(256, 1).astype(np.float32)

    ntA = (NPC + 127) // 128
    g_all = (-FGW * np.log(np.abs(x[:, 127]) + 1e-6)).astype(np.float32)
    iota_np = np.broadcast_to(np.arange(128, dtype=np.float32), (128, 128))
    iota_np = iota_np.astype(ml_dtypes.bfloat16).copy()
    fcw_exp = np.broadcast_to(fc_w.reshape(1, HID), (128, HID)).astype(np.float32).copy()
    fcb_exp = np.full((128, 1), float(fc_b.reshape(-1)[0]), np.float32)
    xT = np.ascontiguousarray(x.T)

    key = (Tm, tuple(TA), tuple(TB))
    if key not in _CACHE:
        _CACHE[key] = _build_program(TA, TB, TT, Tm)
    nc = _CACHE[key]

    W2e = W2e.astype(ml_dtypes.bfloat16)
    W2al = W2al.astype(ml_dtypes.bfloat16)
    in_maps = []
    for c in range(NC):
        pc = per_core[c]
        in_maps.append({
            "W1e": W1e, "W2e": W2e, "W2al": W2al,
            "fcw": fcw_exp, "fcb": fcb_exp, "iota": iota_np,
            "idxm": np.ascontiguousarray(pc["idxm"]),
            "idxal": np.ascontiguousarray(pc["idxal"]),
            "doffm": np.ascontiguousarray(pc["doffm"]),
            "xTl": np.ascontiguousarray(xT[:, c * NPC:(c + 1) * NPC]),
            "gl": np.ascontiguousarray(
                np.pad(g_all[c * NPC:(c + 1) * NPC],
                       (0, ntA * 128 - NPC)).reshape(ntA, 128).T),
        })
    return in_maps, nc


def kernel(x, edge_index, W1, att_l1, att_r1, W2, att_l2, att_r2, fc_w, fc_b):
    x = np.asarray(x, np.float32)
    src = np.asarray(edge_index[0], np.int64)
    dst = np.asarray(edge_index[1], np.int64)
    W1 = np.asarray(W1, np.float32)
    W2 = np.asarray(W2, np.float32)
    al1_ = np.asarray(att_l1, np.float32).reshape(HEADS, HID)
    ar1_ = np.asarray(att_r1, np.float32).reshape(HEADS, HID)
    al2_ = np.asarray(att_l2, np.float32).reshape(1, HID)
    ar2_ = np.asarray(att_r2, np.float32).reshape(1, HID)
    fc_w = np.asarray(fc_w, np.float32)
    fc_b = np.asarray(fc_b, np.float32)

    try:
        in_maps, nc = prep_all(x, src, dst, W1, al1_, ar1_, W2, al2_, ar2_,
                               fc_w, fc_b)
    except Exception:
        return _np_kernel(x, src, dst, W1, al1_, ar1_, W2, al2_, ar2_, fc_w, fc_b)

    def _run():
        res = run_bass_kernel_spmd(nc, in_maps, list(range(NC)))
        return np.concatenate([res.results[c]["yout"] for c in range(NC)], axis=0)

    try:
        return _run()
    except Exception:
        pass
    # The axon worker dies for minutes after any device crash (possibly from a
    # previous job); a tiny computation blocks until it recovers. Probe, then
    # retry once before resorting to the slow numpy fallback.
    try:
        import time as _time
        import jax as _jax
        import jax.numpy as _jnp
        deadline = _time.time() + 300
        while _time.time() < deadline:
            try:
                r = _jax.jit(lambda a: a @ a)(_jnp.ones((8, 8), _jnp.float32))
                r.block_until_ready()
                break
            except Exception:
                _time.sleep(15)
        return _run()
    except Exception:
        return _np_kernel(x, src, dst, W1, al1_, ar1_, W2, al2_, ar2_, fc_w, fc_b)


def _np_gat(xv, src, dst, W, attl, attr, heads, ch):
    n = xv.shape[0]
    h = (xv @ W).reshape(n, heads, ch)
    al = (h * attl).sum(-1)
    ar = (h * attr).sum(-1)
    rw = 1.0 / (np.abs(xv[:, -1:]) + 1e-6)
    rw = rw / np.maximum(np.sum(np.abs(rw), axis=0), 1e-12)
    a = al[dst] + ar[src]
    a = np.where(a > 0, a, NEG * a) + FGW * np.log(rw[src])
    order = np.argsort(dst, kind="stable")
    so, do, ao = src[order], dst[order], a[order]
    seg = np.flatnonzero(np.r_[True, do[1:] != do[:-1]])
    segd = do[seg]
    mx = np.maximum.reduceat(ao, seg, axis=0)
    mfull = np.zeros((n, heads), ao.dtype)
    mfull[segd] = mx
    e = np.exp(ao - mfull[do])
    ssum = np.add.reduceat(e, seg, axis=0)
    sfull = np.zeros((n, heads), e.dtype)
    sfull[segd] = ssum
    alpha = e / (sfull[do] + 1e-16)
    msg = h[so] * alpha[:, :, None]
    acc = np.add.reduceat(msg.reshape(len(so), -1), seg, axis=0)
    out = np.zeros((n, heads * ch), msg.dtype)
    out[segd] = acc
    return out.reshape(n, heads, ch)


def _np_kernel(x, src, dst, W1, al1_, ar1_, W2, al2_, ar2_, fc_w, fc_b):
    x = x.astype(np.float64)
    h = _np_gat(x, src, dst, W1.astype(np.float64), al1_.astype(np.float64),
                ar1_.astype(np.float64), HEADS, HID).reshape(N, HEADS * HID)
    h = np.where(h > 0, h, np.exp(np.minimum(h, 0)) - 1)
    h = _np_gat(h, src, dst, W2.astype(np.float64), al2_.astype(np.float64),
                ar2_.astype(np.float64), 1, HID).reshape(N, HID)
    h = np.where(h > 0, h, np.exp(np.minimum(h, 0)) - 1)
    return (h @ fc_w.astype(np.float64) + fc_b.astype(np.float64)).astype(np.float32)
